# revision 9
# baseline (speedup 1.0000x reference)
"""Multi-head attention layer (B=4, T=S=2048, D=512, H=8) on 8 Trainium2 cores.

Sharding: pure data-parallel over (batch, T-half): core c computes batch c//2,
query rows [1024*(c%2) ...). Weights replicated; no collectives.

The execution runtime this targets has large per-instruction dispatch cost
(~15-50us/instr when streaming) and much larger cross-engine dependency-hop
latency (~100-250us per serialized producer->consumer edge), so the kernel
balances instruction economy with deep per-engine pipelining:
  - All DRAM traffic is large contiguous transfers; 4-byte-element strided
    gathers (descriptor-bound, catastrophic here) are never used.
  - Transposed layouts (x.T, W.T) are built on-chip with regular f32r
    matmuls against an identity, 8 per double-buffered [128, 1024] PSUM
    tile, one evacuation copy each; loads are 512-row groups so DMA,
    PE transposes, DVE copies and projection matmuls pipeline.
  - All matmuls run in float32r (full PE rate, ~2e-4 rounding), moving
    N<=512 (ISA cap for all dtypes).
  - Attention is computed transposed (S.T = K_h @ Q_h.T); softmax exp is a
    single ACT op per s-chunk ([128, 1024] from a double-buffered PSUM
    tile) with scale=1/sqrt(HD) folded in; P.T feeds P@V directly.
  - Softmax denominators ride as a ones-column appended to V per head;
    normalization happens on the small O tensor. No max-subtraction:
    logits are ~N(0,1), exp is safe in fp32.
  - P@V accumulates st=0..15 into one PSUM chain per t-block, trailing the
    S.T/exp stream by LOOK chunks so PE never waits on ACT.
  - bk is dropped entirely: softmax over s is invariant to the per-t
    constant q.bk term, so k-bias cannot affect the output.
"""

from contextlib import ExitStack

import numpy as np

import concourse.bass as bass
import concourse.tile as tile
from concourse import bacc, mybir
from concourse.bass_utils import run_bass_kernel_spmd
from concourse.masks import make_identity

F32 = mybir.dt.float32
F32R = mybir.dt.float32r
AF = mybir.ActivationFunctionType
OP = mybir.AluOpType

B, T, S, D, H = 4, 2048, 2048, 512, 8
HD = D // H          # 64
NCORES = 8
TSH = T // 2         # 1024 query rows per core
DC = D // 128        # 4 contraction chunks
ST = S // 128        # 16 key/value s-tiles
SCALE = 1.0 / np.sqrt(HD)
LOOK = 6             # P@V trails S.T by this many s-chunk slots


def build_nc(repeats: int = 1, stages: str = "lpao"):
    MOV = 512  # matmul moving-operand element cap (all dtypes)

    nc = bacc.Bacc("TRN2", target_bir_lowering=False, debug=False)

    q_d = nc.dram_tensor("q", [TSH, D], F32, kind="ExternalInput")
    k_d = nc.dram_tensor("k", [S, D], F32, kind="ExternalInput")
    v_d = nc.dram_tensor("v", [S, D], F32, kind="ExternalInput")
    w_d = {n: nc.dram_tensor(n, [D, D], F32, kind="ExternalInput")
           for n in ("wq", "wk", "wv", "wo")}
    b_d = {n: nc.dram_tensor(n, [D], F32, kind="ExternalInput")
           for n in ("bq", "bk", "bv", "bo")}
    out_d = nc.dram_tensor("out", [TSH, D], F32, kind="ExternalOutput")

    with tile.TileContext(nc) as tc, ExitStack() as top:
        const = top.enter_context(tc.tile_pool(name="const", bufs=1))
        wt = top.enter_context(tc.tile_pool(name="wt", bufs=1))
        big = top.enter_context(tc.tile_pool(name="big", bufs=1))
        stp = top.enter_context(tc.tile_pool(name="stps", bufs=2, space="PSUM"))

        # ---- constants (once, not per repeat) ----
        ones_f = const.tile([128, 64], F32)
        nc.vector.memset(ones_f, 1.0)
        ones_r = const.tile([128, 64], F32R)
        nc.vector.tensor_copy(ones_r, ones_f)

        bq_sb = const.tile([128, DC], F32)
        bv_bc = const.tile([128, D], F32)
        bo_bc = const.tile([128, D], F32)

        # ---- persistent tensors (f32r) ----
        wts = {"wo": wt.tile([128, DC, D], F32R, tag="wt_wo", name="wt_wo")}
        vp = big.tile([128, ST, H, HD + 1], F32R, tag="vp", name="vp")
        kt = big.tile([128, DC, S], F32R, tag="kt", name="kt")
        qt = big.tile([128, DC, TSH], F32R, tag="qt", name="qt")

        ones_a = const.tile([128, ST * H], F32)
        nc.vector.memset(ones_a, 1.0)
        nc.vector.tensor_copy(
            vp[:, :, :, HD:HD + 1],
            ones_a.rearrange("p (s h o) -> p s h o", s=ST, h=H))

        for _rep in range(repeats):
            # ======== load + project phase (scoped pools) ========
            with ExitStack() as ph:
                loadp = ph.enter_context(tc.tile_pool(name="load", bufs=2))
                xtp = ph.enter_context(tc.tile_pool(name="xtp", bufs=2))
                lconst = ph.enter_context(tc.tile_pool(name="lconst", bufs=1))
                wtp = ph.enter_context(tc.tile_pool(name="wtp", bufs=1))
                tp = ph.enter_context(
                    tc.tile_pool(name="tp", bufs=2, space="PSUM"))
                for n in ("wq", "wk", "wv"):
                    wts[n] = wtp.tile([128, DC, D], F32R, tag=f"wt_{n}",
                                      name=f"wt_{n}")

                ident_f = lconst.tile([128, 128], F32)
                make_identity(nc, ident_f)
                ident_r = lconst.tile([128, 128], F32R)
                nc.vector.tensor_copy(ident_r, ident_f)
                ones1_f = lconst.tile([1, 128], F32)
                nc.vector.memset(ones1_f, 1.0)

                def transpose8(nat, rb0, c0, ps):
                    """8 transposes: row-blocks rb0..rb0+3 x chunks c0..c0+1
                    into one [128, 1024] psum tile, laid out (c, a, t)."""
                    for ci in range(2):
                        for a in range(4):
                            nc.tensor.matmul(
                                ps[:, (ci * 4 + a) * 128:(ci * 4 + a + 1) * 128],
                                nat[:, rb0 + a, 128 * (c0 + ci):128 * (c0 + ci + 1)],
                                ident_r, start=True, stop=True)

                def transpose_group(nat, rb0, xT, t0, name):
                    """nat row-blocks rb0..rb0+3 (512 rows) -> xT[:, :, t0:t0+512]."""
                    for half in range(2):
                        ps = tp.tile([128, 1024], F32, tag="tp",
                                     name=f"tp_{name}{half}")
                        transpose8(nat, rb0, 2 * half, ps)
                        nc.vector.tensor_copy(
                            xT[:, 2 * half:2 * half + 2, t0:t0 + 512],
                            ps.rearrange("p (c t) -> p c t", c=2))

                if "l" in stages:
                    # transposed weights: 1 DMA + 16 transpose-mms + 2 copies
                    for n in ("wk", "wq", "wv", "wo"):
                        wnat = loadp.tile([128, 4, D], F32R, tag="xr",
                                          name=f"wnat_{n}")
                        nc.sync.dma_start(
                            out=wnat,
                            in_=w_d[n].ap()
                            .rearrange("(a p) i -> p a i", p=128)
                            .bitcast(F32R))
                        wview = wts[n].rearrange("p c o -> p c o")
                        transpose_group(wnat, 0, wview, 0, f"w_{n}")

                    # biases bq / bv / bo (bk mathematically irrelevant)
                    b1 = {}
                    for n in ("bq", "bv", "bo"):
                        b1[n] = lconst.tile([1, D], F32, tag="bias",
                                            name=f"b1_{n}")
                        nc.sync.dma_start(
                            out=b1[n],
                            in_=b_d[n].ap().rearrange("(o i) -> o i", o=1))
                    psb = stp.tile([128, DC], F32, tag="st", name="psb")
                    for c in range(DC):
                        nc.tensor.matmul(
                            psb[:, c:c + 1],
                            b1["bq"][0:1, 128 * c:128 * (c + 1)],
                            ones1_f[0:1, 0:1], start=True, stop=True)
                    nc.vector.tensor_copy(bq_sb, psb)
                    for bname, btile in (("bv", bv_bc), ("bo", bo_bc)):
                        psc = stp.tile([128, D], F32, tag="st",
                                       name=f"bc_{bname}")
                        nc.tensor.matmul(psc, ones1_f, b1[bname][0:1, :],
                                         start=True, stop=True)
                        nc.vector.tensor_copy(btile, psc)

                def load_group(dram_view_g, name):
                    """Contiguous 512-row group -> xT [128, DC, 512]."""
                    nat = loadp.tile([128, 4, D], F32R, tag="xr",
                                     name=f"nat_{name}")
                    nc.sync.dma_start(out=nat, in_=dram_view_g.bitcast(F32R))
                    xT = xtp.tile([128, DC, 512], F32R, tag="xt",
                                  name=f"xt_{name}")
                    transpose_group(nat, 0, xT, 0, name)
                    return xT

                if "p" in stages:
                    # V' projection -> vp[s-part, st, h, hd]
                    vview = v_d.ap().rearrange("(g a p) i -> p g a i",
                                               a=4, p=128)
                    for g in range(4):
                        vT = load_group(vview[:, g], f"v{g}")
                        for a in range(4):
                            st = 4 * g + a
                            mm = stp.tile([128, D], F32, tag="st",
                                          name=f"vmm{st}")
                            for c in range(DC):
                                nc.tensor.matmul(
                                    mm, vT[:, c, 128 * a:128 * (a + 1)],
                                    wts["wv"][:, c, :],
                                    start=(c == 0), stop=(c == DC - 1))
                            nc.vector.tensor_tensor(
                                out=vp[:, st, :, 0:HD],
                                in0=mm.rearrange("p (h d) -> p h d", h=H),
                                in1=bv_bc.rearrange("p (h d) -> p h d", h=H),
                                op=OP.add)

                    # K'.T / Q'.T projections -> kt / qt [o-part, c, cols]
                    kview = k_d.ap().rearrange("(g a p) i -> p g a i",
                                               a=4, p=128)
                    qview = q_d.ap().rearrange("(g a p) i -> p g a i",
                                               a=4, p=128)
                    for wname, view, dst, ngroups, bias in (
                        ("wk", kview, kt, 4, None), ("wq", qview, qt, 2, bq_sb),
                    ):
                        for g in range(ngroups):
                            xT = load_group(view[:, g], f"{wname}{g}")
                            for m in range(DC):
                                mm = stp.tile([128, MOV], F32, tag="st",
                                              name="pmm")
                                for c in range(DC):
                                    nc.tensor.matmul(
                                        mm,
                                        wts[wname][:, c, 128 * m:128 * (m + 1)],
                                        xT[:, c, :],
                                        start=(c == 0), stop=(c == DC - 1))
                                if bias is None:
                                    nc.vector.tensor_copy(
                                        dst[:, m, MOV * g:MOV * (g + 1)], mm)
                                else:
                                    nc.vector.tensor_scalar_add(
                                        dst[:, m, MOV * g:MOV * (g + 1)], mm,
                                        bias[:, m:m + 1])

            # ======== attention + output projection (scoped pools) ========
            with ExitStack() as ph:
                attnp = ph.enter_context(tc.tile_pool(name="attnp", bufs=1))
                ptp = ph.enter_context(tc.tile_pool(name="pt", bufs=10))
                rtp = ph.enter_context(tc.tile_pool(name="rt", bufs=2))
                ovp = ph.enter_context(tc.tile_pool(name="ov", bufs=2))
                pvp = ph.enter_context(
                    tc.tile_pool(name="pvps", bufs=4, space="PSUM"))

                ot = attnp.tile([128, DC, TSH], F32R, tag="ot", name="ot")
                outsb = attnp.tile([128, TSH // 128, D], F32, tag="outsb",
                                   name="outsb")
                n_tb = TSH // MOV

                for h in range(H if ("a" in stages or "s" in stages) else 0):
                    ch, pr = h // 2, 64 * (h % 2)
                    pts = {}
                    # one 16-long PSUM accumulation chain per t-block
                    pvs = {tb: pvp.tile([HD + 1, MOV], F32, tag="pv",
                                        name=f"pv{h}_{tb}")
                           for tb in range(n_tb)}

                    def pv_chunk(st):
                        pt = pts.pop(st)
                        for tb in range(n_tb):
                            nc.tensor.matmul(
                                pvs[tb], vp[:, st, h, :],
                                pt[:, MOV * tb:MOV * (tb + 1)],
                                start=(st == 0), stop=(st == ST - 1))

                    for st in range(ST):
                        sps = stp.tile([128, TSH], F32, tag="st",
                                       name=f"sps{h}_{st}")
                        for tb in range(n_tb):
                            nc.tensor.matmul(
                                sps[:, MOV * tb:MOV * (tb + 1)],
                                kt[pr:pr + 64, ch, 128 * st:128 * (st + 1)],
                                qt[pr:pr + 64, ch, MOV * tb:MOV * (tb + 1)],
                                start=True, stop=True)
                        pt = ptp.tile([128, TSH], F32R, tag="pt",
                                      name=f"pt{h}_{st}")
                        nc.scalar.activation(pt, sps, AF.Exp,
                                             scale=float(SCALE))
                        pts[st] = pt
                        if "a" in stages and st >= LOOK:
                            pv_chunk(st - LOOK)
                    if "a" not in stages:
                        pts.clear()
                        continue
                    for st in range(ST - LOOK, ST):
                        pv_chunk(st)

                    # normalize by the denominator row
                    for tb in range(n_tb):
                        cmb = ovp.tile([HD + 1, MOV], F32, tag="ov",
                                       name=f"cb{h}_{tb}")
                        nc.vector.tensor_copy(cmb, pvs[tb])
                        rt = rtp.tile([65, MOV], F32R, tag="rt",
                                      name=f"rt{h}_{tb}")
                        with nc.allow_low_precision(
                                reason="recip feeds f32r matmul"):
                            nc.vector.reciprocal(rt[64:65, :],
                                                 cmb[HD:HD + 1, :])
                        bc = pvp.tile([64, MOV], F32, tag="pv",
                                      name=f"bcm{h}_{tb}")
                        nc.tensor.matmul(bc, ones_r[64:65, :], rt[64:65, :],
                                         start=True, stop=True)
                        nc.vector.tensor_tensor(
                            out=ot[pr:pr + 64, ch, MOV * tb:MOV * (tb + 1)],
                            in0=cmb[0:HD, :], in1=bc, op=OP.mult)

                # ---- output projection (into one staging tile, 1 store) ----
                for tt in range(TSH // 128 if "o" in stages else 0):
                    mm = stp.tile([128, D], F32, tag="st", name=f"omm{tt}")
                    for c in range(DC):
                        nc.tensor.matmul(mm, ot[:, c, 128 * tt:128 * (tt + 1)],
                                         wts["wo"][:, c, :],
                                         start=(c == 0), stop=(c == DC - 1))
                    nc.vector.tensor_tensor(out=outsb[:, tt, :], in0=mm,
                                            in1=bo_bc, op=OP.add)
                if "o" in stages:
                    nc.sync.dma_start(
                        out=out_d.ap().rearrange("(a p) i -> p a i", p=128),
                        in_=outsb)

    nc.compile()
    return nc


_CACHE = {}


def _get_nc():
    if "nc" not in _CACHE:
        _CACHE["nc"] = build_nc()
    return _CACHE["nc"]


def kernel(query, key, value, Wq, bq, Wk, bk, Wv, bv, Wo, bo):
    f = lambda x: np.ascontiguousarray(np.asarray(x, dtype=np.float32))
    query, key, value = f(query), f(key), f(value)
    shared = {"wq": f(Wq), "wk": f(Wk), "wv": f(Wv), "wo": f(Wo),
              "bq": f(bq), "bk": f(bk), "bv": f(bv), "bo": f(bo)}
    in_maps = []
    for c in range(NCORES):
        b, th = divmod(c, 2)
        in_maps.append({
            "q": query[b, th * TSH:(th + 1) * TSH, :],
            "k": key[b], "v": value[b], **shared,
        })
    nc = _get_nc()
    res = run_bass_kernel_spmd(nc, in_maps, core_ids=list(range(NCORES)))
    out = np.empty((B, T, D), dtype=np.float32)
    for c in range(NCORES):
        b, th = divmod(c, 2)
        out[b, th * TSH:(th + 1) * TSH, :] = res.results[c]["out"]
    return out


# revision 11
# speedup vs baseline: 1.0574x; 1.0574x over previous
"""Multi-head attention layer (B=4, T=S=2048, D=512, H=8) on 8 Trainium2 cores.

Sharding: pure data-parallel over (batch, T-half): core c computes batch c//2,
query rows [1024*(c%2) ...). Weights replicated; no collectives.

The execution runtime this targets charges a large, roughly-uniform cost PER
INSTRUCTION (measured ~50us/matmul, ~100us/PE-transpose, ~200us/activation,
~300us/DMA, ~55us/DVE op — nearly independent of operand size), so the kernel
is built around strict instruction economy:
  - All DRAM traffic is large contiguous transfers (one DMA per tensor);
    4-byte-element strided gathers (descriptor-bound) are never used.
  - Transposed layouts (x.T, W.T) are built on-chip with REGULAR f32r
    matmuls against an identity (cheaper per instruction than is_transpose
    on this runtime), 16 transposes per 4-bank PSUM tile, one evacuation
    copy per 2048 columns.
  - All compute matmuls run in float32r (fp32-width, full PE rate, ~2e-4
    rounding) with fp32 PSUM accumulation, moving N=512.
  - Attention is computed transposed (S.T = K_h @ Q_h.T); softmax exp is ACT
    over TWO s-chunks at once ([128, 2048] from a 4-bank PSUM tile), with
    scale=1/sqrt(HD) folded in; P.T feeds P@V directly with no transposes.
  - Softmax denominators ride as a ones-column appended to V per head (row 64
    of the PV accumulation); normalization happens on the small O tensor.
    No max-subtraction: logits are ~N(0,1), exp is safe in fp32.
  - P@V accumulates st=0..15 into a single PSUM group per t-block.
  - Biases are staged through 1-partition contiguous loads and broadcast /
    transposed with K=1 outer-product matmuls (plain f32: odd/narrow outputs
    violate fp32r ISA restrictions).
"""

from contextlib import ExitStack

import numpy as np

import concourse.bass as bass
import concourse.tile as tile
from concourse import bacc, mybir
from concourse.bass_utils import run_bass_kernel_spmd
from concourse.masks import make_identity

F32 = mybir.dt.float32
F32R = mybir.dt.float32r
AF = mybir.ActivationFunctionType
OP = mybir.AluOpType

B, T, S, D, H = 4, 2048, 2048, 512, 8
HD = D // H          # 64
NCORES = 8
TSH = T // 2         # 1024 query rows per core
DC = D // 128        # 4 contraction chunks
ST = S // 128        # 16 key/value s-tiles
SCALE = 1.0 / np.sqrt(HD)
LOOK = 6             # P@V trails S.T by this many s-chunk slots


def build_nc(repeats: int = 1, stages: str = "lpao"):
    MOV = 512  # f32r matmul N limit

    nc = bacc.Bacc("TRN2", target_bir_lowering=False, debug=False)

    q_d = nc.dram_tensor("q", [TSH, D], F32, kind="ExternalInput")
    k_d = nc.dram_tensor("k", [S, D], F32, kind="ExternalInput")
    v_d = nc.dram_tensor("v", [S, D], F32, kind="ExternalInput")
    w_d = {n: nc.dram_tensor(n, [D, D], F32, kind="ExternalInput")
           for n in ("wq", "wk", "wv", "wo")}
    b_d = {n: nc.dram_tensor(n, [D], F32, kind="ExternalInput")
           for n in ("bq", "bk", "bv", "bo")}
    out_d = nc.dram_tensor("out", [TSH, D], F32, kind="ExternalOutput")

    with tile.TileContext(nc) as tc, ExitStack() as top:
        const = top.enter_context(tc.tile_pool(name="const", bufs=1))
        wt = top.enter_context(tc.tile_pool(name="wt", bufs=1))
        big = top.enter_context(tc.tile_pool(name="big", bufs=1))
        # single 4-bank PSUM slot, shared by everything on the main path
        stp = top.enter_context(tc.tile_pool(name="stps", bufs=1, space="PSUM"))

        # ---- constants (once, not per repeat) ----
        ones_f = const.tile([128, 64], F32)
        nc.vector.memset(ones_f, 1.0)
        ones_r = const.tile([128, 64], F32R)
        nc.vector.tensor_copy(ones_r, ones_f)

        bq_sb = const.tile([128, DC], F32)
        bv_bc = const.tile([128, D], F32)
        bo_bc = const.tile([128, D], F32)

        # ---- persistent tensors (f32r) ----
        wts = {"wo": wt.tile([128, DC, D], F32R, tag="wt_wo", name="wt_wo")}
        vp = big.tile([128, ST, H, HD + 1], F32R, tag="vp", name="vp")
        kt = big.tile([128, DC, S], F32R, tag="kt", name="kt")
        qt = big.tile([128, DC, TSH], F32R, tag="qt", name="qt")

        ones_a = const.tile([128, ST * H], F32)
        nc.vector.memset(ones_a, 1.0)
        nc.vector.tensor_copy(
            vp[:, :, :, HD:HD + 1],
            ones_a.rearrange("p (s h o) -> p s h o", s=ST, h=H))

        for _rep in range(repeats):
            # ======== load + project phase (scoped pools) ========
            with ExitStack() as ph:
                loadp = ph.enter_context(tc.tile_pool(name="load", bufs=1))
                xtp = ph.enter_context(tc.tile_pool(name="xtp", bufs=1))
                lconst = ph.enter_context(tc.tile_pool(name="lconst", bufs=1))
                wtp = ph.enter_context(tc.tile_pool(name="wtp", bufs=1))
                tp = ph.enter_context(
                    tc.tile_pool(name="tp", bufs=1, space="PSUM"))
                for n in ("wq", "wk", "wv"):
                    wts[n] = wtp.tile([128, DC, D], F32R, tag=f"wt_{n}",
                                      name=f"wt_{n}")

                ident_f = lconst.tile([128, 128], F32)
                make_identity(nc, ident_f)
                ident_r = lconst.tile([128, 128], F32R)
                nc.vector.tensor_copy(ident_r, ident_f)
                ones1_f = lconst.tile([1, 128], F32)
                nc.vector.memset(ones1_f, 1.0)

                def transpose16(nat, rb0, ps):
                    """16 transposes: nat row-blocks rb0..rb0+3 x 4 chunks
                    into one [128, 2048] psum tile, laid out (c, a, t)."""
                    for c in range(DC):
                        for a in range(4):
                            nc.tensor.matmul(
                                ps[:, (c * 4 + a) * 128:(c * 4 + a + 1) * 128],
                                nat[:, rb0 + a, 128 * c:128 * (c + 1)],
                                ident_r, start=True, stop=True)

                if "l" in stages:
                    # transposed weights: 1 DMA + 16 transpose-mms + 1 copy
                    for n in ("wk", "wq", "wv", "wo"):
                        wnat = loadp.tile([128, 4, D], F32R, tag="xr",
                                          name=f"wnat_{n}")
                        nc.sync.dma_start(
                            out=wnat,
                            in_=w_d[n].ap()
                            .rearrange("(a p) i -> p a i", p=128)
                            .bitcast(F32R))
                        ps = tp.tile([128, 2048], F32, tag="tp",
                                     name=f"wtp_{n}")
                        transpose16(wnat, 0, ps)
                        nc.vector.tensor_copy(
                            wts[n].rearrange("p c o -> p (c o)"), ps)

                    # biases (plain f32 outer products; fp32r ISA disallows
                    # odd/narrow outputs)
                    # bk is dropped entirely: softmax over s is invariant
                    # to the per-t constant q.bk logit term.
                    b1 = {}
                    for n in ("bq", "bv", "bo"):
                        b1[n] = lconst.tile([1, D], F32, tag="bias",
                                            name=f"b1_{n}")
                        nc.sync.dma_start(
                            out=b1[n],
                            in_=b_d[n].ap().rearrange("(o i) -> o i", o=1))
                    psb = stp.tile([128, DC], F32, tag="st", name="psb")
                    for c in range(DC):
                        nc.tensor.matmul(
                            psb[:, c:c + 1],
                            b1["bq"][0:1, 128 * c:128 * (c + 1)],
                            ones1_f[0:1, 0:1], start=True, stop=True)
                    nc.vector.tensor_copy(bq_sb, psb)
                    for bname, btile in (("bv", bv_bc), ("bo", bo_bc)):
                        psc = stp.tile([128, D], F32, tag="st",
                                       name=f"bc_{bname}")
                        nc.tensor.matmul(psc, ones1_f, b1[bname][0:1, :],
                                         start=True, stop=True)
                        nc.vector.tensor_copy(btile, psc)

                def load_xt(dram, n_rb, name):
                    """Whole tensor: 1 DMA, then 16-transpose groups into
                    xT [128, DC, 128*n_rb] (layout [i-part, c, rows])."""
                    nat = loadp.tile([128, n_rb, D], F32R, tag="xr",
                                     name=f"nat_{name}")
                    nc.sync.dma_start(
                        out=nat,
                        in_=dram.ap().rearrange("(a p) i -> p a i", p=128)
                        .bitcast(F32R))
                    xT = xtp.tile([128, DC, 128 * n_rb], F32R, tag="xt",
                                  name=f"xt_{name}")
                    for g in range(n_rb // 4):
                        ps = tp.tile([128, 2048], F32, tag="tp",
                                     name=f"tp_{name}{g}")
                        transpose16(nat, 4 * g, ps)
                        nc.vector.tensor_copy(
                            xT[:, :, 512 * g:512 * (g + 1)], ps)
                    return xT

                if "p" in stages:
                    # V' projection -> vp[s-part, st, h, hd]
                    vT = load_xt(v_d, ST, "v")
                    for st in range(ST):
                        mm = stp.tile([128, D], F32, tag="st", name=f"vmm{st}")
                        for c in range(DC):
                            nc.tensor.matmul(
                                mm, vT[:, c, 128 * st:128 * (st + 1)],
                                wts["wv"][:, c, :],
                                start=(c == 0), stop=(c == DC - 1))
                        nc.vector.tensor_tensor(
                            out=vp[:, st, :, 0:HD],
                            in0=mm.rearrange("p (h d) -> p h d", h=H),
                            in1=bv_bc.rearrange("p (h d) -> p h d", h=H),
                            op=OP.add)

                    # K'.T / Q'.T projections -> kt / qt [o-part, c, cols]
                    for wname, dram, dst, ncols, bias in (
                        ("wk", k_d, kt, S, None),
                        ("wq", q_d, qt, TSH, bq_sb),
                    ):
                        xT = load_xt(dram, ncols // 128, wname[1])
                        for g in range(ncols // MOV):
                            for m in range(DC):
                                mm = stp.tile([128, MOV], F32, tag="st",
                                              name="pmm")
                                for c in range(DC):
                                    nc.tensor.matmul(
                                        mm,
                                        wts[wname][:, c, 128 * m:128 * (m + 1)],
                                        xT[:, c, MOV * g:MOV * (g + 1)],
                                        start=(c == 0), stop=(c == DC - 1))
                                if bias is None:
                                    nc.vector.tensor_copy(
                                        dst[:, m, MOV * g:MOV * (g + 1)], mm)
                                else:
                                    nc.vector.tensor_scalar_add(
                                        dst[:, m, MOV * g:MOV * (g + 1)], mm,
                                        bias[:, m:m + 1])

            # ======== attention + output projection (scoped pools) ========
            with ExitStack() as ph:
                attnp = ph.enter_context(tc.tile_pool(name="attnp", bufs=1))
                ptp = ph.enter_context(tc.tile_pool(name="pt", bufs=4))
                rtp = ph.enter_context(tc.tile_pool(name="rt", bufs=2))
                ovp = ph.enter_context(tc.tile_pool(name="ov", bufs=2))
                pvp = ph.enter_context(
                    tc.tile_pool(name="pvps", bufs=2, space="PSUM"))

                ot = attnp.tile([128, DC, TSH], F32R, tag="ot", name="ot")
                outsb = attnp.tile([128, TSH // 128, D], F32, tag="outsb",
                                   name="outsb")
                n_tb = TSH // MOV

                for h in range(H if ("a" in stages or "s" in stages) else 0):
                    ch, pr = h // 2, 64 * (h % 2)
                    pts = {}
                    # one 16-long PSUM accumulation chain per t-block,
                    # both chains in one 2-bank tile for a merged combine
                    pvs = pvp.tile([HD + 1, TSH], F32, tag="pv",
                                   name=f"pv{h}")

                    def pv_chunk(st):
                        pt, base = pts.pop(st)
                        for tb in range(n_tb):
                            nc.tensor.matmul(
                                pvs[:, MOV * tb:MOV * (tb + 1)],
                                vp[:, st, h, :],
                                pt[:, base + MOV * tb:base + MOV * (tb + 1)],
                                start=(st == 0), stop=(st == ST - 1))

                    nxt = 0
                    for pi in range(ST // 2):
                        sps = stp.tile([128, 2048], F32, tag="st",
                                       name=f"sps{h}_{pi}")
                        for sub in range(2):
                            st = 2 * pi + sub
                            for tb in range(n_tb):
                                nc.tensor.matmul(
                                    sps[:, 1024 * sub + MOV * tb:
                                        1024 * sub + MOV * (tb + 1)],
                                    kt[pr:pr + 64, ch, 128 * st:128 * (st + 1)],
                                    qt[pr:pr + 64, ch, MOV * tb:MOV * (tb + 1)],
                                    start=True, stop=True)
                        pt = ptp.tile([128, 2048], F32R, tag="pt",
                                      name=f"pt{h}_{pi}")
                        nc.scalar.activation(pt, sps, AF.Exp,
                                             scale=float(SCALE))
                        for sub in range(2):
                            pts[2 * pi + sub] = (pt, 1024 * sub)
                        if "a" in stages:
                            while nxt <= 2 * pi + 1 - LOOK:
                                pv_chunk(nxt)
                                nxt += 1
                    if "a" not in stages:
                        pts.clear()
                        continue
                    while nxt < ST:
                        pv_chunk(nxt)
                        nxt += 1

                    # normalize by the denominator row (merged over t-blocks)
                    cmb = ovp.tile([HD + 1, TSH], F32, tag="ov",
                                   name=f"cb{h}")
                    nc.vector.tensor_copy(cmb, pvs)
                    rt = rtp.tile([65, TSH], F32R, tag="rt", name=f"rt{h}")
                    with nc.allow_low_precision(
                            reason="recip feeds f32r matmul"):
                        nc.vector.reciprocal(rt[64:65, :],
                                             cmb[HD:HD + 1, :])
                    bc = pvp.tile([64, TSH], F32, tag="pv", name=f"bcm{h}")
                    for tb in range(n_tb):
                        nc.tensor.matmul(bc[:, MOV * tb:MOV * (tb + 1)],
                                         ones_r[64:65, :],
                                         rt[64:65, MOV * tb:MOV * (tb + 1)],
                                         start=True, stop=True)
                    nc.vector.tensor_tensor(
                        out=ot[pr:pr + 64, ch, :],
                        in0=cmb[0:HD, :], in1=bc, op=OP.mult)

                # ---- output projection (into one staging tile, 1 store) ----
                for tt in range(TSH // 128 if "o" in stages else 0):
                    mm = stp.tile([128, D], F32, tag="st", name=f"omm{tt}")
                    for c in range(DC):
                        nc.tensor.matmul(mm, ot[:, c, 128 * tt:128 * (tt + 1)],
                                         wts["wo"][:, c, :],
                                         start=(c == 0), stop=(c == DC - 1))
                    nc.vector.tensor_tensor(out=outsb[:, tt, :], in0=mm,
                                            in1=bo_bc, op=OP.add)
                if "o" in stages:
                    nc.sync.dma_start(
                        out=out_d.ap().rearrange("(a p) i -> p a i", p=128),
                        in_=outsb)

    nc.compile()
    return nc


_CACHE = {}


def _get_nc():
    if "nc" not in _CACHE:
        _CACHE["nc"] = build_nc()
    return _CACHE["nc"]


def kernel(query, key, value, Wq, bq, Wk, bk, Wv, bv, Wo, bo):
    f = lambda x: np.ascontiguousarray(np.asarray(x, dtype=np.float32))
    query, key, value = f(query), f(key), f(value)
    shared = {"wq": f(Wq), "wk": f(Wk), "wv": f(Wv), "wo": f(Wo),
              "bq": f(bq), "bk": f(bk), "bv": f(bv), "bo": f(bo)}
    in_maps = []
    for c in range(NCORES):
        b, th = divmod(c, 2)
        in_maps.append({
            "q": query[b, th * TSH:(th + 1) * TSH, :],
            "k": key[b], "v": value[b], **shared,
        })
    nc = _get_nc()
    res = run_bass_kernel_spmd(nc, in_maps, core_ids=list(range(NCORES)))
    out = np.empty((B, T, D), dtype=np.float32)
    for c in range(NCORES):
        b, th = divmod(c, 2)
        out[b, th * TSH:(th + 1) * TSH, :] = res.results[c]["out"]
    return out


# revision 12
# speedup vs baseline: 1.1957x; 1.1307x over previous
"""Multi-head attention layer (B=4, T=S=2048, D=512, H=8) on 8 Trainium2 cores.

Sharding: pure data-parallel over (batch, T-half): core c computes batch c//2,
query rows [1024*(c%2) ...). Weights replicated; no collectives.

The execution runtime this targets charges a large, roughly-uniform cost PER
INSTRUCTION (measured ~50us/matmul, ~100us/PE-transpose, ~200us/activation,
~300us/DMA, ~55us/DVE op — nearly independent of operand size), so the kernel
is built around strict instruction economy:
  - All DRAM traffic is large contiguous transfers (one DMA per tensor);
    4-byte-element strided gathers (descriptor-bound) are never used.
  - Transposed layouts (x.T, W.T) are built on-chip with REGULAR f32r
    matmuls against an identity (cheaper per instruction than is_transpose
    on this runtime), 16 transposes per 4-bank PSUM tile, one evacuation
    copy per 2048 columns.
  - All compute matmuls run in float32r (fp32-width, full PE rate, ~2e-4
    rounding) with fp32 PSUM accumulation, moving N=512.
  - Attention is computed transposed (S.T = K_h @ Q_h.T); softmax exp is ACT
    over TWO s-chunks at once ([128, 2048] from a 4-bank PSUM tile), with
    scale=1/sqrt(HD) folded in; P.T feeds P@V directly with no transposes.
  - Softmax denominators ride as a ones-column appended to V per head (row 64
    of the PV accumulation); normalization happens on the small O tensor.
    No max-subtraction: logits are ~N(0,1), exp is safe in fp32.
  - P@V accumulates st=0..15 into a single PSUM group per t-block.
  - Biases are staged through 1-partition contiguous loads and broadcast /
    transposed with K=1 outer-product matmuls (plain f32: odd/narrow outputs
    violate fp32r ISA restrictions).
"""

from contextlib import ExitStack

import numpy as np

import concourse.bass as bass
import concourse.tile as tile
from concourse import bacc, mybir
from concourse.bass_utils import run_bass_kernel_spmd
from concourse.masks import make_identity

F32 = mybir.dt.float32
F32R = mybir.dt.float32r
AF = mybir.ActivationFunctionType
OP = mybir.AluOpType

B, T, S, D, H = 4, 2048, 2048, 512, 8
HD = D // H          # 64
NCORES = 8
TSH = T // 2         # 1024 query rows per core
DC = D // 128        # 4 contraction chunks
ST = S // 128        # 16 key/value s-tiles
SCALE = 1.0 / np.sqrt(HD)
LOOK = 6             # P@V trails S.T by this many s-chunk slots


def build_nc(repeats: int = 1, stages: str = "lpao"):
    MOV = 512  # f32r matmul N limit

    nc = bacc.Bacc("TRN2", target_bir_lowering=False, debug=False)

    q_d = nc.dram_tensor("q", [TSH, D], F32, kind="ExternalInput")
    k_d = nc.dram_tensor("k", [S, D], F32, kind="ExternalInput")
    v_d = nc.dram_tensor("v", [S, D], F32, kind="ExternalInput")
    w_d = {n: nc.dram_tensor(n, [D, D], F32, kind="ExternalInput")
           for n in ("wq", "wk", "wv", "wo")}
    b_d = {n: nc.dram_tensor(n, [D], F32, kind="ExternalInput")
           for n in ("bq", "bk", "bv", "bo")}
    out_d = nc.dram_tensor("out", [TSH, D], F32, kind="ExternalOutput")

    with tile.TileContext(nc) as tc, ExitStack() as top:
        const = top.enter_context(tc.tile_pool(name="const", bufs=1))
        wt = top.enter_context(tc.tile_pool(name="wt", bufs=1))
        big = top.enter_context(tc.tile_pool(name="big", bufs=1))
        # single 4-bank PSUM slot, shared by everything on the main path
        stp = top.enter_context(tc.tile_pool(name="stps", bufs=1, space="PSUM"))

        # ---- constants (once, not per repeat) ----
        ones_f = const.tile([128, 64], F32)
        nc.vector.memset(ones_f, 1.0)
        ones_r = const.tile([128, 64], F32R)
        nc.vector.tensor_copy(ones_r, ones_f)

        bq_sb = const.tile([128, DC], F32)
        bk_sb = const.tile([128, DC], F32)
        bv_bc = const.tile([128, D], F32)
        bo_bc = const.tile([128, D], F32)

        # ---- persistent tensors (f32r) ----
        wts = {"wo": wt.tile([128, DC, D], F32R, tag="wt_wo", name="wt_wo")}
        vp = big.tile([128, ST, H, HD + 1], F32R, tag="vp", name="vp")
        kt = big.tile([128, DC, S], F32R, tag="kt", name="kt")
        qt = big.tile([128, DC, TSH], F32R, tag="qt", name="qt")

        ones_a = const.tile([128, ST * H], F32)
        nc.vector.memset(ones_a, 1.0)
        nc.vector.tensor_copy(
            vp[:, :, :, HD:HD + 1],
            ones_a.rearrange("p (s h o) -> p s h o", s=ST, h=H))

        for _rep in range(repeats):
            # ======== load + project phase (scoped pools) ========
            with ExitStack() as ph:
                loadp = ph.enter_context(tc.tile_pool(name="load", bufs=1))
                xtp = ph.enter_context(tc.tile_pool(name="xtp", bufs=1))
                lconst = ph.enter_context(tc.tile_pool(name="lconst", bufs=1))
                wtp = ph.enter_context(tc.tile_pool(name="wtp", bufs=1))
                tp = ph.enter_context(
                    tc.tile_pool(name="tp", bufs=1, space="PSUM"))
                for n in ("wq", "wk", "wv"):
                    wts[n] = wtp.tile([128, DC, D], F32R, tag=f"wt_{n}",
                                      name=f"wt_{n}")

                ident_f = lconst.tile([128, 128], F32)
                make_identity(nc, ident_f)
                ident_r = lconst.tile([128, 128], F32R)
                nc.vector.tensor_copy(ident_r, ident_f)
                ones1_f = lconst.tile([1, 128], F32)
                nc.vector.memset(ones1_f, 1.0)

                def transpose16(nat, rb0, ps):
                    """16 transposes: nat row-blocks rb0..rb0+3 x 4 chunks
                    into one [128, 2048] psum tile, laid out (c, a, t)."""
                    for c in range(DC):
                        for a in range(4):
                            nc.tensor.matmul(
                                ps[:, (c * 4 + a) * 128:(c * 4 + a + 1) * 128],
                                nat[:, rb0 + a, 128 * c:128 * (c + 1)],
                                ident_r, start=True, stop=True)

                if "l" in stages:
                    # transposed weights: 1 DMA + 16 transpose-mms + 1 copy
                    for n in ("wk", "wq", "wv", "wo"):
                        wnat = loadp.tile([128, 4, D], F32R, tag="xr",
                                          name=f"wnat_{n}")
                        nc.sync.dma_start(
                            out=wnat,
                            in_=w_d[n].ap()
                            .rearrange("(a p) i -> p a i", p=128)
                            .bitcast(F32R))
                        ps = tp.tile([128, 2048], F32, tag="tp",
                                     name=f"wtp_{n}")
                        transpose16(wnat, 0, ps)
                        nc.vector.tensor_copy(
                            wts[n].rearrange("p c o -> p (c o)"), ps)

                    # biases (plain f32 outer products; fp32r ISA disallows
                    # odd/narrow outputs)
                    b1 = {}
                    for n in ("bq", "bk", "bv", "bo"):
                        b1[n] = lconst.tile([1, D], F32, tag="bias",
                                            name=f"b1_{n}")
                        nc.sync.dma_start(
                            out=b1[n],
                            in_=b_d[n].ap().rearrange("(o i) -> o i", o=1))
                    psb = stp.tile([128, 2 * DC], F32, tag="st", name="psb")
                    for j, bname in ((0, "bq"), (1, "bk")):
                        for c in range(DC):
                            nc.tensor.matmul(
                                psb[:, j * DC + c:j * DC + c + 1],
                                b1[bname][0:1, 128 * c:128 * (c + 1)],
                                ones1_f[0:1, 0:1], start=True, stop=True)
                    nc.vector.tensor_copy(bq_sb, psb[:, 0:DC])
                    nc.vector.tensor_copy(bk_sb, psb[:, DC:2 * DC])
                    for bname, btile in (("bv", bv_bc), ("bo", bo_bc)):
                        psc = stp.tile([128, D], F32, tag="st",
                                       name=f"bc_{bname}")
                        nc.tensor.matmul(psc, ones1_f, b1[bname][0:1, :],
                                         start=True, stop=True)
                        nc.vector.tensor_copy(btile, psc)

                def load_xt(dram, n_rb, name):
                    """Whole tensor: 1 DMA, then 16-transpose groups into
                    xT [128, DC, 128*n_rb] (layout [i-part, c, rows])."""
                    nat = loadp.tile([128, n_rb, D], F32R, tag="xr",
                                     name=f"nat_{name}")
                    nc.sync.dma_start(
                        out=nat,
                        in_=dram.ap().rearrange("(a p) i -> p a i", p=128)
                        .bitcast(F32R))
                    xT = xtp.tile([128, DC, 128 * n_rb], F32R, tag="xt",
                                  name=f"xt_{name}")
                    for g in range(n_rb // 4):
                        ps = tp.tile([128, 2048], F32, tag="tp",
                                     name=f"tp_{name}{g}")
                        transpose16(nat, 4 * g, ps)
                        nc.vector.tensor_copy(
                            xT[:, :, 512 * g:512 * (g + 1)], ps)
                    return xT

                if "p" in stages:
                    # V' projection -> vp[s-part, st, h, hd]
                    vT = load_xt(v_d, ST, "v")
                    for st in range(ST):
                        mm = stp.tile([128, D], F32, tag="st", name=f"vmm{st}")
                        for c in range(DC):
                            nc.tensor.matmul(
                                mm, vT[:, c, 128 * st:128 * (st + 1)],
                                wts["wv"][:, c, :],
                                start=(c == 0), stop=(c == DC - 1))
                        nc.vector.tensor_tensor(
                            out=vp[:, st, :, 0:HD],
                            in0=mm.rearrange("p (h d) -> p h d", h=H),
                            in1=bv_bc.rearrange("p (h d) -> p h d", h=H),
                            op=OP.add)

                    # K'.T / Q'.T projections -> kt / qt [o-part, c, cols]
                    for wname, dram, dst, ncols, bias in (
                        ("wk", k_d, kt, S, bk_sb),
                        ("wq", q_d, qt, TSH, bq_sb),
                    ):
                        xT = load_xt(dram, ncols // 128, wname[1])
                        for g in range(ncols // MOV):
                            for m in range(DC):
                                mm = stp.tile([128, MOV], F32, tag="st",
                                              name="pmm")
                                for c in range(DC):
                                    nc.tensor.matmul(
                                        mm,
                                        wts[wname][:, c, 128 * m:128 * (m + 1)],
                                        xT[:, c, MOV * g:MOV * (g + 1)],
                                        start=(c == 0), stop=(c == DC - 1))
                                nc.vector.tensor_scalar_add(
                                    dst[:, m, MOV * g:MOV * (g + 1)], mm,
                                    bias[:, m:m + 1])

            # ======== attention + output projection (scoped pools) ========
            with ExitStack() as ph:
                attnp = ph.enter_context(tc.tile_pool(name="attnp", bufs=1))
                ptp = ph.enter_context(tc.tile_pool(name="pt", bufs=4))
                rtp = ph.enter_context(tc.tile_pool(name="rt", bufs=2))
                ovp = ph.enter_context(tc.tile_pool(name="ov", bufs=2))
                pvp = ph.enter_context(
                    tc.tile_pool(name="pvps", bufs=4, space="PSUM"))

                ot = attnp.tile([128, DC, TSH], F32R, tag="ot", name="ot")
                outsb = attnp.tile([128, TSH // 128, D], F32, tag="outsb",
                                   name="outsb")
                n_tb = TSH // MOV

                for h in range(H if ("a" in stages or "s" in stages) else 0):
                    ch, pr = h // 2, 64 * (h % 2)
                    pts = {}
                    # one 16-long PSUM accumulation chain per t-block
                    pvs = {tb: pvp.tile([HD + 1, MOV], F32, tag="pv",
                                        name=f"pv{h}_{tb}")
                           for tb in range(n_tb)}

                    def pv_chunk(st):
                        pt, base = pts.pop(st)
                        for tb in range(n_tb):
                            nc.tensor.matmul(
                                pvs[tb], vp[:, st, h, :],
                                pt[:, base + MOV * tb:base + MOV * (tb + 1)],
                                start=(st == 0), stop=(st == ST - 1))

                    nxt = 0
                    for pi in range(ST // 2):
                        sps = stp.tile([128, 2048], F32, tag="st",
                                       name=f"sps{h}_{pi}")
                        for sub in range(2):
                            st = 2 * pi + sub
                            for tb in range(n_tb):
                                nc.tensor.matmul(
                                    sps[:, 1024 * sub + MOV * tb:
                                        1024 * sub + MOV * (tb + 1)],
                                    kt[pr:pr + 64, ch, 128 * st:128 * (st + 1)],
                                    qt[pr:pr + 64, ch, MOV * tb:MOV * (tb + 1)],
                                    start=True, stop=True)
                        pt = ptp.tile([128, 2048], F32R, tag="pt",
                                      name=f"pt{h}_{pi}")
                        nc.scalar.activation(pt, sps, AF.Exp,
                                             scale=float(SCALE))
                        for sub in range(2):
                            pts[2 * pi + sub] = (pt, 1024 * sub)
                        if "a" in stages:
                            while nxt <= 2 * pi + 1 - LOOK:
                                pv_chunk(nxt)
                                nxt += 1
                    if "a" not in stages:
                        pts.clear()
                        continue
                    while nxt < ST:
                        pv_chunk(nxt)
                        nxt += 1

                    # normalize by the denominator row
                    for tb in range(n_tb):
                        cmb = ovp.tile([HD + 1, MOV], F32, tag="ov",
                                       name=f"cb{h}_{tb}")
                        nc.vector.tensor_copy(cmb, pvs[tb])
                        rt = rtp.tile([65, MOV], F32R, tag="rt",
                                      name=f"rt{h}_{tb}")
                        with nc.allow_low_precision(
                                reason="recip feeds f32r matmul"):
                            nc.vector.reciprocal(rt[64:65, :],
                                                 cmb[HD:HD + 1, :])
                        bc = pvp.tile([64, MOV], F32, tag="pv",
                                      name=f"bcm{h}_{tb}")
                        nc.tensor.matmul(bc, ones_r[64:65, :], rt[64:65, :],
                                         start=True, stop=True)
                        nc.vector.tensor_tensor(
                            out=ot[pr:pr + 64, ch, MOV * tb:MOV * (tb + 1)],
                            in0=cmb[0:HD, :], in1=bc, op=OP.mult)

                # ---- output projection (into one staging tile, 1 store) ----
                for tt in range(TSH // 128 if "o" in stages else 0):
                    mm = stp.tile([128, D], F32, tag="st", name=f"omm{tt}")
                    for c in range(DC):
                        nc.tensor.matmul(mm, ot[:, c, 128 * tt:128 * (tt + 1)],
                                         wts["wo"][:, c, :],
                                         start=(c == 0), stop=(c == DC - 1))
                    nc.vector.tensor_tensor(out=outsb[:, tt, :], in0=mm,
                                            in1=bo_bc, op=OP.add)
                if "o" in stages:
                    nc.sync.dma_start(
                        out=out_d.ap().rearrange("(a p) i -> p a i", p=128),
                        in_=outsb)

    nc.compile()
    return nc


_CACHE = {}


def _get_nc():
    if "nc" not in _CACHE:
        _CACHE["nc"] = build_nc()
    return _CACHE["nc"]


def kernel(query, key, value, Wq, bq, Wk, bk, Wv, bv, Wo, bo):
    f = lambda x: np.ascontiguousarray(np.asarray(x, dtype=np.float32))
    query, key, value = f(query), f(key), f(value)
    shared = {"wq": f(Wq), "wk": f(Wk), "wv": f(Wv), "wo": f(Wo),
              "bq": f(bq), "bk": f(bk), "bv": f(bv), "bo": f(bo)}
    in_maps = []
    for c in range(NCORES):
        b, th = divmod(c, 2)
        in_maps.append({
            "q": query[b, th * TSH:(th + 1) * TSH, :],
            "k": key[b], "v": value[b], **shared,
        })
    nc = _get_nc()
    res = run_bass_kernel_spmd(nc, in_maps, core_ids=list(range(NCORES)))
    out = np.empty((B, T, D), dtype=np.float32)
    for c in range(NCORES):
        b, th = divmod(c, 2)
        out[b, th * TSH:(th + 1) * TSH, :] = res.results[c]["out"]
    return out


# revision 16
# speedup vs baseline: 1.1975x; 1.0016x over previous
"""Multi-head attention layer (B=4, T=S=2048, D=512, H=8) on 8 Trainium2 cores.

Sharding: pure data-parallel over (batch, T-half): core c computes batch c//2,
query rows [1024*(c%2) ...). Weights replicated; no collectives.

The execution runtime this targets charges a large, roughly-uniform cost PER
INSTRUCTION (measured ~50us/matmul, ~100us/PE-transpose, ~200us/activation,
~300us/DMA, ~55us/DVE op — nearly independent of operand size), so the kernel
is built around strict instruction economy:
  - All DRAM traffic is large contiguous transfers (one DMA per tensor);
    4-byte-element strided gathers (descriptor-bound) are never used.
  - Transposed layouts (x.T, W.T) are built on-chip with REGULAR f32r
    matmuls against an identity (cheaper per instruction than is_transpose
    on this runtime), 16 transposes per 4-bank PSUM tile, one evacuation
    copy per 2048 columns.
  - All compute matmuls run in float32r (fp32-width, full PE rate, ~2e-4
    rounding) with fp32 PSUM accumulation, moving N=512.
  - Attention is computed transposed (S.T = K_h @ Q_h.T); softmax exp is ACT
    over TWO s-chunks at once ([128, 2048] from a 4-bank PSUM tile), with
    scale=1/sqrt(HD) folded in; P.T feeds P@V directly with no transposes.
  - Softmax denominators ride as a ones-column appended to V per head (row 64
    of the PV accumulation); normalization happens on the small O tensor.
    No max-subtraction: logits are ~N(0,1), exp is safe in fp32.
  - P@V accumulates st=0..15 into a single PSUM group per t-block.
  - Biases are staged through 1-partition contiguous loads and broadcast /
    transposed with K=1 outer-product matmuls (plain f32: odd/narrow outputs
    violate fp32r ISA restrictions).
"""

from contextlib import ExitStack

import numpy as np

import concourse.bass as bass
import concourse.tile as tile
from concourse import bacc, mybir
from concourse.bass_utils import run_bass_kernel_spmd
from concourse.masks import make_identity

F32 = mybir.dt.float32
F32R = mybir.dt.float32r
AF = mybir.ActivationFunctionType
OP = mybir.AluOpType

B, T, S, D, H = 4, 2048, 2048, 512, 8
HD = D // H          # 64
NCORES = 8
TSH = T // 2         # 1024 query rows per core
DC = D // 128        # 4 contraction chunks
ST = S // 128        # 16 key/value s-tiles
SCALE = 1.0 / np.sqrt(HD)
LOOK = 6             # P@V trails S.T by this many s-chunk slots


def build_nc(repeats: int = 1, stages: str = "lpao", v_overlap: bool = False):
    MOV = 512  # f32r matmul N limit

    nc = bacc.Bacc("TRN2", target_bir_lowering=False, debug=False)

    q_d = nc.dram_tensor("q", [TSH, D], F32, kind="ExternalInput")
    k_d = nc.dram_tensor("k", [S, D], F32, kind="ExternalInput")
    v_d = nc.dram_tensor("v", [S, D], F32, kind="ExternalInput")
    w_d = {n: nc.dram_tensor(n, [D, D], F32, kind="ExternalInput")
           for n in ("wq", "wk", "wv", "wo")}
    b_d = {n: nc.dram_tensor(n, [D], F32, kind="ExternalInput")
           for n in ("bq", "bk", "bv", "bo")}
    out_d = nc.dram_tensor("out", [TSH, D], F32, kind="ExternalOutput")

    with tile.TileContext(nc) as tc, ExitStack() as top:
        const = top.enter_context(tc.tile_pool(name="const", bufs=1))
        wt = top.enter_context(tc.tile_pool(name="wt", bufs=1))
        big = top.enter_context(tc.tile_pool(name="big", bufs=1))
        # single 4-bank PSUM slot, shared by everything on the main path
        stp = top.enter_context(tc.tile_pool(name="stps", bufs=1, space="PSUM"))

        # ---- constants (once, not per repeat) ----
        ones_f = const.tile([128, 64], F32)
        nc.vector.memset(ones_f, 1.0)
        ones_r = const.tile([128, 64], F32R)
        nc.vector.tensor_copy(ones_r, ones_f)

        bq_sb = const.tile([128, DC], F32)
        bk_sb = const.tile([128, DC], F32)
        bv_bc = const.tile([128, D], F32)
        bo_bc = const.tile([128, D], F32)

        # ---- persistent tensors (f32r) ----
        wts = {"wo": wt.tile([128, DC, D], F32R, tag="wt_wo", name="wt_wo")}
        ident_r2 = None
        if v_overlap:
            # wv must outlive the load scope: its projection runs inside the
            # attention scope, overlapped with the exp stream
            wts["wv"] = wt.tile([128, DC, D], F32R, tag="wt_wv", name="wt_wv")
            ident_f2 = const.tile([128, 128], F32)
            make_identity(nc, ident_f2)
            ident_r2 = const.tile([128, 128], F32R)
            nc.vector.tensor_copy(ident_r2, ident_f2)
        vp = big.tile([128, ST, H, HD + 1], F32R, tag="vp", name="vp")
        kt = big.tile([128, DC, S], F32R, tag="kt", name="kt")
        qt = big.tile([128, DC, TSH], F32R, tag="qt", name="qt")

        ones_a = const.tile([128, ST * H], F32)
        nc.vector.memset(ones_a, 1.0)
        nc.vector.tensor_copy(
            vp[:, :, :, HD:HD + 1],
            ones_a.rearrange("p (s h o) -> p s h o", s=ST, h=H))

        for _rep in range(repeats):
            # ======== load + project phase (scoped pools) ========
            with ExitStack() as ph:
                loadp = ph.enter_context(tc.tile_pool(name="load", bufs=1))
                xtp = ph.enter_context(tc.tile_pool(name="xtp", bufs=1))
                lconst = ph.enter_context(tc.tile_pool(name="lconst", bufs=1))
                wtp = ph.enter_context(tc.tile_pool(name="wtp", bufs=1))
                tp = ph.enter_context(
                    tc.tile_pool(name="tp", bufs=1, space="PSUM"))
                for n in (("wq", "wk") if v_overlap else ("wq", "wk", "wv")):
                    wts[n] = wtp.tile([128, DC, D], F32R, tag=f"wt_{n}",
                                      name=f"wt_{n}")

                ident_f = lconst.tile([128, 128], F32)
                make_identity(nc, ident_f)
                ident_r = lconst.tile([128, 128], F32R)
                nc.vector.tensor_copy(ident_r, ident_f)
                ones1_f = lconst.tile([1, 128], F32)
                nc.vector.memset(ones1_f, 1.0)

                def transpose16(nat, rb0, ps):
                    """16 transposes: nat row-blocks rb0..rb0+3 x 4 chunks
                    into one [128, 2048] psum tile, laid out (c, a, t)."""
                    for c in range(DC):
                        for a in range(4):
                            nc.tensor.matmul(
                                ps[:, (c * 4 + a) * 128:(c * 4 + a + 1) * 128],
                                nat[:, rb0 + a, 128 * c:128 * (c + 1)],
                                ident_r, start=True, stop=True)

                if "l" in stages:
                    # transposed weights: 1 DMA + 16 transpose-mms + 1 copy
                    for n in ("wk", "wq", "wv", "wo"):
                        wnat = loadp.tile([128, 4, D], F32R, tag="xr",
                                          name=f"wnat_{n}")
                        nc.sync.dma_start(
                            out=wnat,
                            in_=w_d[n].ap()
                            .rearrange("(a p) i -> p a i", p=128)
                            .bitcast(F32R))
                        ps = tp.tile([128, 2048], F32, tag="tp",
                                     name=f"wtp_{n}")
                        transpose16(wnat, 0, ps)
                        nc.vector.tensor_copy(
                            wts[n].rearrange("p c o -> p (c o)"), ps)

                    # biases (plain f32 outer products; fp32r ISA disallows
                    # odd/narrow outputs)
                    b1 = {}
                    for n in ("bq", "bk", "bv", "bo"):
                        b1[n] = lconst.tile([1, D], F32, tag="bias",
                                            name=f"b1_{n}")
                        nc.sync.dma_start(
                            out=b1[n],
                            in_=b_d[n].ap().rearrange("(o i) -> o i", o=1))
                    psb = stp.tile([128, 2 * DC], F32, tag="st", name="psb")
                    for j, bname in ((0, "bq"), (1, "bk")):
                        for c in range(DC):
                            nc.tensor.matmul(
                                psb[:, j * DC + c:j * DC + c + 1],
                                b1[bname][0:1, 128 * c:128 * (c + 1)],
                                ones1_f[0:1, 0:1], start=True, stop=True)
                    nc.vector.tensor_copy(bq_sb, psb[:, 0:DC])
                    nc.vector.tensor_copy(bk_sb, psb[:, DC:2 * DC])
                    for bname, btile in (("bv", bv_bc), ("bo", bo_bc)):
                        psc = stp.tile([128, D], F32, tag="st",
                                       name=f"bc_{bname}")
                        nc.tensor.matmul(psc, ones1_f, b1[bname][0:1, :],
                                         start=True, stop=True)
                        nc.vector.tensor_copy(btile, psc)

                def load_xt(dram, n_rb, name):
                    """Whole tensor: 1 DMA, then 16-transpose groups into
                    xT [128, DC, 128*n_rb] (layout [i-part, c, rows])."""
                    nat = loadp.tile([128, n_rb, D], F32R, tag="xr",
                                     name=f"nat_{name}")
                    nc.sync.dma_start(
                        out=nat,
                        in_=dram.ap().rearrange("(a p) i -> p a i", p=128)
                        .bitcast(F32R))
                    xT = xtp.tile([128, DC, 128 * n_rb], F32R, tag="xt",
                                  name=f"xt_{name}")
                    for g in range(n_rb // 4):
                        ps = tp.tile([128, 2048], F32, tag="tp",
                                     name=f"tp_{name}{g}")
                        transpose16(nat, 4 * g, ps)
                        nc.vector.tensor_copy(
                            xT[:, :, 512 * g:512 * (g + 1)], ps)
                    return xT

                if "p" in stages and not v_overlap:
                    # V' projection -> vp[s-part, st, h, hd]
                    vT = load_xt(v_d, ST, "v")
                    for st in range(ST):
                        mm = stp.tile([128, D], F32, tag="st", name=f"vmm{st}")
                        for c in range(DC):
                            nc.tensor.matmul(
                                mm, vT[:, c, 128 * st:128 * (st + 1)],
                                wts["wv"][:, c, :],
                                start=(c == 0), stop=(c == DC - 1))
                        nc.vector.tensor_tensor(
                            out=vp[:, st, :, 0:HD],
                            in0=mm.rearrange("p (h d) -> p h d", h=H),
                            in1=bv_bc.rearrange("p (h d) -> p h d", h=H),
                            op=OP.add)
                if "p" in stages:

                    # K'.T / Q'.T projections -> kt / qt [o-part, c, cols]
                    for wname, dram, dst, ncols, bias in (
                        ("wk", k_d, kt, S, bk_sb),
                        ("wq", q_d, qt, TSH, bq_sb),
                    ):
                        xT = load_xt(dram, ncols // 128, wname[1])
                        for g in range(ncols // MOV):
                            for m in range(DC):
                                mm = stp.tile([128, MOV], F32, tag="st",
                                              name="pmm")
                                for c in range(DC):
                                    nc.tensor.matmul(
                                        mm,
                                        wts[wname][:, c, 128 * m:128 * (m + 1)],
                                        xT[:, c, MOV * g:MOV * (g + 1)],
                                        start=(c == 0), stop=(c == DC - 1))
                                nc.vector.tensor_scalar_add(
                                    dst[:, m, MOV * g:MOV * (g + 1)], mm,
                                    bias[:, m:m + 1])

            # ======== attention + output projection (scoped pools) ========
            with ExitStack() as ph:
                attnp = ph.enter_context(tc.tile_pool(name="attnp", bufs=1))
                ptp = ph.enter_context(tc.tile_pool(name="pt", bufs=4))
                rtp = ph.enter_context(
                    tc.tile_pool(name="rt", bufs=1 if v_overlap else 2))
                ovp = ph.enter_context(
                    tc.tile_pool(name="ov", bufs=1 if v_overlap else 2))
                pvp = ph.enter_context(tc.tile_pool(
                    name="pvps", bufs=2 if v_overlap else 4, space="PSUM"))
                if v_overlap:
                    vload = ph.enter_context(tc.tile_pool(name="vload", bufs=2))
                    vtp = ph.enter_context(tc.tile_pool(name="vtp", bufs=2))
                    aux = ph.enter_context(
                        tc.tile_pool(name="aux", bufs=1, space="PSUM"))

                    # V pipeline, overlapped with the head loop's exp stream:
                    # runs through its own 2-bank PSUM slot so it never
                    # contends with the QK/exp slot.
                    vview = v_d.ap().rearrange("(g a p) i -> p g a i",
                                               a=4, p=128)
                    if "p" in stages:
                        for g in range(4):
                            nat = vload.tile([128, 4, D], F32R, tag="vnat",
                                             name=f"vnat{g}")
                            nc.sync.dma_start(out=nat,
                                              in_=vview[:, g].bitcast(F32R))
                            vT = vtp.tile([128, DC, 512], F32R, tag="vt",
                                          name=f"vt{g}")
                            for half in range(2):
                                ps = aux.tile([128, 1024], F32, tag="aux",
                                              name=f"vtp{g}_{half}")
                                for ci in range(2):
                                    for a in range(4):
                                        nc.tensor.matmul(
                                            ps[:, (ci * 4 + a) * 128:
                                               (ci * 4 + a + 1) * 128],
                                            nat[:, a, 128 * (2 * half + ci):
                                                128 * (2 * half + ci + 1)],
                                            ident_r2, start=True, stop=True)
                                nc.vector.tensor_copy(
                                    vT[:, 2 * half:2 * half + 2, :],
                                    ps.rearrange("p (c t) -> p c t", c=2))
                            for a in range(4):
                                st = 4 * g + a
                                mm = aux.tile([128, D], F32, tag="aux",
                                              name=f"vmm{st}")
                                for c in range(DC):
                                    nc.tensor.matmul(
                                        mm, vT[:, c, 128 * a:128 * (a + 1)],
                                        wts["wv"][:, c, :],
                                        start=(c == 0), stop=(c == DC - 1))
                                nc.vector.tensor_tensor(
                                    out=vp[:, st, :, 0:HD],
                                    in0=mm.rearrange("p (h d) -> p h d", h=H),
                                    in1=bv_bc.rearrange("p (h d) -> p h d",
                                                        h=H),
                                    op=OP.add)

                ot = attnp.tile([128, DC, TSH], F32R, tag="ot", name="ot")
                outsb = attnp.tile([128, TSH // 128, D], F32, tag="outsb",
                                   name="outsb")
                n_tb = TSH // MOV

                for h in range(H if ("a" in stages or "s" in stages) else 0):
                    ch, pr = h // 2, 64 * (h % 2)
                    pts = {}
                    # one 16-long PSUM accumulation chain per t-block
                    pvs = {tb: pvp.tile([HD + 1, MOV], F32, tag="pv",
                                        name=f"pv{h}_{tb}")
                           for tb in range(n_tb)}

                    def pv_chunk(st):
                        pt, base = pts.pop(st)
                        for tb in range(n_tb):
                            nc.tensor.matmul(
                                pvs[tb], vp[:, st, h, :],
                                pt[:, base + MOV * tb:base + MOV * (tb + 1)],
                                start=(st == 0), stop=(st == ST - 1))

                    nxt = 0
                    for pi in range(ST // 2):
                        sps = stp.tile([128, 2048], F32, tag="st",
                                       name=f"sps{h}_{pi}")
                        for sub in range(2):
                            st = 2 * pi + sub
                            for tb in range(n_tb):
                                nc.tensor.matmul(
                                    sps[:, 1024 * sub + MOV * tb:
                                        1024 * sub + MOV * (tb + 1)],
                                    kt[pr:pr + 64, ch, 128 * st:128 * (st + 1)],
                                    qt[pr:pr + 64, ch, MOV * tb:MOV * (tb + 1)],
                                    start=True, stop=True)
                        pt = ptp.tile([128, 2048], F32R, tag="pt",
                                      name=f"pt{h}_{pi}")
                        nc.scalar.activation(pt, sps, AF.Exp,
                                             scale=float(SCALE))
                        for sub in range(2):
                            pts[2 * pi + sub] = (pt, 1024 * sub)
                        if "a" in stages:
                            while nxt <= 2 * pi + 1 - LOOK:
                                pv_chunk(nxt)
                                nxt += 1
                    if "a" not in stages:
                        pts.clear()
                        continue
                    while nxt < ST:
                        pv_chunk(nxt)
                        nxt += 1

                    # normalize by the denominator row
                    for tb in range(n_tb):
                        cmb = ovp.tile([HD + 1, MOV], F32, tag="ov",
                                       name=f"cb{h}_{tb}")
                        nc.vector.tensor_copy(cmb, pvs[tb])
                        rt = rtp.tile([65, MOV], F32R, tag="rt",
                                      name=f"rt{h}_{tb}")
                        with nc.allow_low_precision(
                                reason="recip feeds f32r matmul"):
                            nc.vector.reciprocal(rt[64:65, :],
                                                 cmb[HD:HD + 1, :])
                        bc = (aux if v_overlap else pvp).tile(
                            [64, MOV], F32, tag="aux" if v_overlap else "pv",
                            name=f"bcm{h}_{tb}")
                        nc.tensor.matmul(bc, ones_r[64:65, :], rt[64:65, :],
                                         start=True, stop=True)
                        nc.vector.tensor_tensor(
                            out=ot[pr:pr + 64, ch, MOV * tb:MOV * (tb + 1)],
                            in0=cmb[0:HD, :], in1=bc, op=OP.mult)

                # ---- output projection (into one staging tile, 1 store) ----
                for tt in range(TSH // 128 if "o" in stages else 0):
                    mm = stp.tile([128, D], F32, tag="st", name=f"omm{tt}")
                    for c in range(DC):
                        nc.tensor.matmul(mm, ot[:, c, 128 * tt:128 * (tt + 1)],
                                         wts["wo"][:, c, :],
                                         start=(c == 0), stop=(c == DC - 1))
                    nc.vector.tensor_tensor(out=outsb[:, tt, :], in0=mm,
                                            in1=bo_bc, op=OP.add)
                if "o" in stages:
                    nc.sync.dma_start(
                        out=out_d.ap().rearrange("(a p) i -> p a i", p=128),
                        in_=outsb)

    nc.compile()
    return nc


_CACHE = {}


def _get_nc():
    if "nc" not in _CACHE:
        _CACHE["nc"] = build_nc()
    return _CACHE["nc"]


def kernel(query, key, value, Wq, bq, Wk, bk, Wv, bv, Wo, bo):
    f = lambda x: np.ascontiguousarray(np.asarray(x, dtype=np.float32))
    query, key, value = f(query), f(key), f(value)
    shared = {"wq": f(Wq), "wk": f(Wk), "wv": f(Wv), "wo": f(Wo),
              "bq": f(bq), "bk": f(bk), "bv": f(bv), "bo": f(bo)}
    in_maps = []
    for c in range(NCORES):
        b, th = divmod(c, 2)
        in_maps.append({
            "q": query[b, th * TSH:(th + 1) * TSH, :],
            "k": key[b], "v": value[b], **shared,
        })
    nc = _get_nc()
    res = run_bass_kernel_spmd(nc, in_maps, core_ids=list(range(NCORES)))
    out = np.empty((B, T, D), dtype=np.float32)
    for c in range(NCORES):
        b, th = divmod(c, 2)
        out[b, th * TSH:(th + 1) * TSH, :] = res.results[c]["out"]
    return out


# revision 18
# speedup vs baseline: 1.2971x; 1.0831x over previous
"""Multi-head attention layer (B=4, T=S=2048, D=512, H=8) on 8 Trainium2 cores.

Sharding: pure data-parallel over (batch, T-half): core c computes batch c//2,
query rows [1024*(c%2) ...). Weights replicated; no collectives.

The execution runtime this targets charges a large, roughly-uniform cost PER
INSTRUCTION (measured ~50us/matmul, ~100us/PE-transpose, ~200us/activation,
~300us/DMA, ~55us/DVE op — nearly independent of operand size), so the kernel
is built around strict instruction economy:
  - All DRAM traffic is large contiguous transfers (one DMA per tensor);
    4-byte-element strided gathers (descriptor-bound) are never used.
  - Transposed layouts (x.T, W.T) are built on-chip with REGULAR f32r
    matmuls against an identity (cheaper per instruction than is_transpose
    on this runtime), 16 transposes per 4-bank PSUM tile, one evacuation
    copy per 2048 columns.
  - All compute matmuls run in float32r (fp32-width, full PE rate, ~2e-4
    rounding) with fp32 PSUM accumulation, moving N=512.
  - Attention is computed transposed (S.T = K_h @ Q_h.T); softmax exp is ACT
    over TWO s-chunks at once ([128, 2048] from a 4-bank PSUM tile), with
    scale=1/sqrt(HD) folded in; P.T feeds P@V directly with no transposes.
  - Softmax denominators ride as a ones-column appended to V per head (row 64
    of the PV accumulation); normalization happens on the small O tensor.
    No max-subtraction: logits are ~N(0,1), exp is safe in fp32.
  - P@V accumulates st=0..15 into a single PSUM group per t-block.
  - Biases are staged through 1-partition contiguous loads and broadcast /
    transposed with K=1 outer-product matmuls (plain f32: odd/narrow outputs
    violate fp32r ISA restrictions).
"""

from contextlib import ExitStack

import numpy as np

import concourse.bass as bass
import concourse.tile as tile
from concourse import bacc, mybir
from concourse.bass_utils import run_bass_kernel_spmd
from concourse.masks import make_identity

F32 = mybir.dt.float32
F32R = mybir.dt.float32r
AF = mybir.ActivationFunctionType
OP = mybir.AluOpType

B, T, S, D, H = 4, 2048, 2048, 512, 8
HD = D // H          # 64
NCORES = 8
TSH = T // 2         # 1024 query rows per core
DC = D // 128        # 4 contraction chunks
ST = S // 128        # 16 key/value s-tiles
SCALE = 1.0 / np.sqrt(HD)
LOOK = 6             # P@V trails S.T by this many s-chunk slots


def build_nc(repeats: int = 1, stages: str = "lpao", v_overlap: bool = False,
             fast_exp: bool = False):
    MOV = 512  # f32r matmul N limit

    nc = bacc.Bacc("TRN2", target_bir_lowering=False, debug=False)

    q_d = nc.dram_tensor("q", [TSH, D], F32, kind="ExternalInput")
    k_d = nc.dram_tensor("k", [S, D], F32, kind="ExternalInput")
    v_d = nc.dram_tensor("v", [S, D], F32, kind="ExternalInput")
    w_d = {n: nc.dram_tensor(n, [D, D], F32, kind="ExternalInput")
           for n in ("wq", "wk", "wv", "wo")}
    b_d = {n: nc.dram_tensor(n, [D], F32, kind="ExternalInput")
           for n in ("bq", "bk", "bv", "bo")}
    out_d = nc.dram_tensor("out", [TSH, D], F32, kind="ExternalOutput")

    with tile.TileContext(nc) as tc, ExitStack() as top:
        const = top.enter_context(tc.tile_pool(name="const", bufs=1))
        wt = top.enter_context(tc.tile_pool(name="wt", bufs=1))
        big = top.enter_context(tc.tile_pool(name="big", bufs=1))
        # single 4-bank PSUM slot, shared by everything on the main path
        stp = top.enter_context(tc.tile_pool(name="stps", bufs=1, space="PSUM"))

        # ---- constants (once, not per repeat) ----
        ones_f = const.tile([128, 64], F32)
        nc.vector.memset(ones_f, 1.0)
        ones_r = const.tile([128, 64], F32R)
        nc.vector.tensor_copy(ones_r, ones_f)

        bq_sb = const.tile([128, DC], F32)
        bk_sb = const.tile([128, DC], F32)
        bv_bc = const.tile([128, D], F32)
        bo_bc = const.tile([128, D], F32)

        # ---- persistent tensors (f32r) ----
        wts = {"wo": wt.tile([128, DC, D], F32R, tag="wt_wo", name="wt_wo")}
        ident_r2 = None
        if v_overlap:
            # wv must outlive the load scope: its projection runs inside the
            # attention scope, overlapped with the exp stream
            wts["wv"] = wt.tile([128, DC, D], F32R, tag="wt_wv", name="wt_wv")
            ident_f2 = const.tile([128, 128], F32)
            make_identity(nc, ident_f2)
            ident_r2 = const.tile([128, 128], F32R)
            nc.vector.tensor_copy(ident_r2, ident_f2)
        vp = big.tile([128, ST, H, HD + 1], F32R, tag="vp", name="vp")
        kt = big.tile([128, DC, S], F32R, tag="kt", name="kt")
        qt = big.tile([128, DC, TSH], F32R, tag="qt", name="qt")

        ones_a = const.tile([128, ST * H], F32)
        nc.vector.memset(ones_a, 1.0)
        nc.vector.tensor_copy(
            vp[:, :, :, HD:HD + 1],
            ones_a.rearrange("p (s h o) -> p s h o", s=ST, h=H))

        for _rep in range(repeats):
            # ======== load + project phase (scoped pools) ========
            with ExitStack() as ph:
                loadp = ph.enter_context(tc.tile_pool(name="load", bufs=1))
                xtp = ph.enter_context(tc.tile_pool(name="xtp", bufs=1))
                lconst = ph.enter_context(tc.tile_pool(name="lconst", bufs=1))
                wtp = ph.enter_context(tc.tile_pool(name="wtp", bufs=1))
                tp = ph.enter_context(
                    tc.tile_pool(name="tp", bufs=1, space="PSUM"))
                for n in (("wq", "wk") if v_overlap else ("wq", "wk", "wv")):
                    wts[n] = wtp.tile([128, DC, D], F32R, tag=f"wt_{n}",
                                      name=f"wt_{n}")

                ident_f = lconst.tile([128, 128], F32)
                make_identity(nc, ident_f)
                ident_r = lconst.tile([128, 128], F32R)
                nc.vector.tensor_copy(ident_r, ident_f)
                ones1_f = lconst.tile([1, 128], F32)
                nc.vector.memset(ones1_f, 1.0)

                def transpose16(nat, rb0, ps):
                    """16 transposes: nat row-blocks rb0..rb0+3 x 4 chunks
                    into one [128, 2048] psum tile, laid out (c, a, t)."""
                    for c in range(DC):
                        for a in range(4):
                            nc.tensor.matmul(
                                ps[:, (c * 4 + a) * 128:(c * 4 + a + 1) * 128],
                                nat[:, rb0 + a, 128 * c:128 * (c + 1)],
                                ident_r, start=True, stop=True)

                if "l" in stages:
                    # transposed weights: 1 DMA + 16 transpose-mms + 1 copy
                    for n in ("wk", "wq", "wv", "wo"):
                        wnat = loadp.tile([128, 4, D], F32R, tag="xr",
                                          name=f"wnat_{n}")
                        nc.sync.dma_start(
                            out=wnat,
                            in_=w_d[n].ap()
                            .rearrange("(a p) i -> p a i", p=128)
                            .bitcast(F32R))
                        ps = tp.tile([128, 2048], F32, tag="tp",
                                     name=f"wtp_{n}")
                        transpose16(wnat, 0, ps)
                        nc.vector.tensor_copy(
                            wts[n].rearrange("p c o -> p (c o)"), ps)

                    # biases (plain f32 outer products; fp32r ISA disallows
                    # odd/narrow outputs)
                    b1 = {}
                    for n in ("bq", "bk", "bv", "bo"):
                        b1[n] = lconst.tile([1, D], F32, tag="bias",
                                            name=f"b1_{n}")
                        nc.sync.dma_start(
                            out=b1[n],
                            in_=b_d[n].ap().rearrange("(o i) -> o i", o=1))
                    psb = stp.tile([128, 2 * DC], F32, tag="st", name="psb")
                    for j, bname in ((0, "bq"), (1, "bk")):
                        for c in range(DC):
                            nc.tensor.matmul(
                                psb[:, j * DC + c:j * DC + c + 1],
                                b1[bname][0:1, 128 * c:128 * (c + 1)],
                                ones1_f[0:1, 0:1], start=True, stop=True)
                    nc.vector.tensor_copy(bq_sb, psb[:, 0:DC])
                    nc.vector.tensor_copy(bk_sb, psb[:, DC:2 * DC])
                    for bname, btile in (("bv", bv_bc), ("bo", bo_bc)):
                        psc = stp.tile([128, D], F32, tag="st",
                                       name=f"bc_{bname}")
                        nc.tensor.matmul(psc, ones1_f, b1[bname][0:1, :],
                                         start=True, stop=True)
                        nc.vector.tensor_copy(btile, psc)

                def load_xt(dram, n_rb, name):
                    """Whole tensor: 1 DMA, then 16-transpose groups into
                    xT [128, DC, 128*n_rb] (layout [i-part, c, rows])."""
                    nat = loadp.tile([128, n_rb, D], F32R, tag="xr",
                                     name=f"nat_{name}")
                    nc.sync.dma_start(
                        out=nat,
                        in_=dram.ap().rearrange("(a p) i -> p a i", p=128)
                        .bitcast(F32R))
                    xT = xtp.tile([128, DC, 128 * n_rb], F32R, tag="xt",
                                  name=f"xt_{name}")
                    for g in range(n_rb // 4):
                        ps = tp.tile([128, 2048], F32, tag="tp",
                                     name=f"tp_{name}{g}")
                        transpose16(nat, 4 * g, ps)
                        nc.vector.tensor_copy(
                            xT[:, :, 512 * g:512 * (g + 1)], ps)
                    return xT

                if "p" in stages and not v_overlap:
                    # V' projection -> vp[s-part, st, h, hd]
                    vT = load_xt(v_d, ST, "v")
                    for st in range(ST):
                        mm = stp.tile([128, D], F32, tag="st", name=f"vmm{st}")
                        for c in range(DC):
                            nc.tensor.matmul(
                                mm, vT[:, c, 128 * st:128 * (st + 1)],
                                wts["wv"][:, c, :],
                                start=(c == 0), stop=(c == DC - 1))
                        nc.vector.tensor_tensor(
                            out=vp[:, st, :, 0:HD],
                            in0=mm.rearrange("p (h d) -> p h d", h=H),
                            in1=bv_bc.rearrange("p (h d) -> p h d", h=H),
                            op=OP.add)
                if "p" in stages:

                    # K'.T / Q'.T projections -> kt / qt [o-part, c, cols]
                    for wname, dram, dst, ncols, bias in (
                        ("wk", k_d, kt, S, bk_sb),
                        ("wq", q_d, qt, TSH, bq_sb),
                    ):
                        xT = load_xt(dram, ncols // 128, wname[1])
                        for g in range(ncols // MOV):
                            for m in range(DC):
                                mm = stp.tile([128, MOV], F32, tag="st",
                                              name="pmm")
                                for c in range(DC):
                                    nc.tensor.matmul(
                                        mm,
                                        wts[wname][:, c, 128 * m:128 * (m + 1)],
                                        xT[:, c, MOV * g:MOV * (g + 1)],
                                        start=(c == 0), stop=(c == DC - 1))
                                nc.vector.tensor_scalar_add(
                                    dst[:, m, MOV * g:MOV * (g + 1)], mm,
                                    bias[:, m:m + 1])

            # ======== attention + output projection (scoped pools) ========
            with ExitStack() as ph:
                attnp = ph.enter_context(tc.tile_pool(name="attnp", bufs=1))
                ptp = ph.enter_context(tc.tile_pool(name="pt", bufs=4))
                rtp = ph.enter_context(
                    tc.tile_pool(name="rt", bufs=1 if v_overlap else 2))
                ovp = ph.enter_context(
                    tc.tile_pool(name="ov", bufs=1 if v_overlap else 2))
                pvp = ph.enter_context(tc.tile_pool(
                    name="pvps", bufs=2 if v_overlap else 4, space="PSUM"))
                if v_overlap:
                    vload = ph.enter_context(tc.tile_pool(name="vload", bufs=2))
                    vtp = ph.enter_context(tc.tile_pool(name="vtp", bufs=2))
                    aux = ph.enter_context(
                        tc.tile_pool(name="aux", bufs=1, space="PSUM"))

                    # V pipeline, overlapped with the head loop's exp stream:
                    # runs through its own 2-bank PSUM slot so it never
                    # contends with the QK/exp slot.
                    vview = v_d.ap().rearrange("(g a p) i -> p g a i",
                                               a=4, p=128)
                    if "p" in stages:
                        for g in range(4):
                            nat = vload.tile([128, 4, D], F32R, tag="vnat",
                                             name=f"vnat{g}")
                            nc.sync.dma_start(out=nat,
                                              in_=vview[:, g].bitcast(F32R))
                            vT = vtp.tile([128, DC, 512], F32R, tag="vt",
                                          name=f"vt{g}")
                            for half in range(2):
                                ps = aux.tile([128, 1024], F32, tag="aux",
                                              name=f"vtp{g}_{half}")
                                for ci in range(2):
                                    for a in range(4):
                                        nc.tensor.matmul(
                                            ps[:, (ci * 4 + a) * 128:
                                               (ci * 4 + a + 1) * 128],
                                            nat[:, a, 128 * (2 * half + ci):
                                                128 * (2 * half + ci + 1)],
                                            ident_r2, start=True, stop=True)
                                nc.vector.tensor_copy(
                                    vT[:, 2 * half:2 * half + 2, :],
                                    ps.rearrange("p (c t) -> p c t", c=2))
                            for a in range(4):
                                st = 4 * g + a
                                mm = aux.tile([128, D], F32, tag="aux",
                                              name=f"vmm{st}")
                                for c in range(DC):
                                    nc.tensor.matmul(
                                        mm, vT[:, c, 128 * a:128 * (a + 1)],
                                        wts["wv"][:, c, :],
                                        start=(c == 0), stop=(c == DC - 1))
                                nc.vector.tensor_tensor(
                                    out=vp[:, st, :, 0:HD],
                                    in0=mm.rearrange("p (h d) -> p h d", h=H),
                                    in1=bv_bc.rearrange("p (h d) -> p h d",
                                                        h=H),
                                    op=OP.add)

                ot = attnp.tile([128, DC, TSH], F32R, tag="ot", name="ot")
                outsb = attnp.tile([128, TSH // 128, D], F32, tag="outsb",
                                   name="outsb")
                n_tb = TSH // MOV

                for h in range(H if ("a" in stages or "s" in stages) else 0):
                    ch, pr = h // 2, 64 * (h % 2)
                    pts = {}
                    # one 16-long PSUM accumulation chain per t-block
                    pvs = {tb: pvp.tile([HD + 1, MOV], F32, tag="pv",
                                        name=f"pv{h}_{tb}")
                           for tb in range(n_tb)}

                    def pv_chunk(st):
                        pt, base = pts.pop(st)
                        for tb in range(n_tb):
                            nc.tensor.matmul(
                                pvs[tb], vp[:, st, h, :],
                                pt[:, base + MOV * tb:base + MOV * (tb + 1)],
                                start=(st == 0), stop=(st == ST - 1))

                    nxt = 0
                    for pi in range(ST // 2):
                        sps = stp.tile([128, 2048], F32, tag="st",
                                       name=f"sps{h}_{pi}")
                        for sub in range(2):
                            st = 2 * pi + sub
                            for tb in range(n_tb):
                                nc.tensor.matmul(
                                    sps[:, 1024 * sub + MOV * tb:
                                        1024 * sub + MOV * (tb + 1)],
                                    kt[pr:pr + 64, ch, 128 * st:128 * (st + 1)],
                                    qt[pr:pr + 64, ch, MOV * tb:MOV * (tb + 1)],
                                    start=True, stop=True)
                        if fast_exp and (pi % 2 == 1):
                            # Schraudolph fast exp on DVE: reinterpret
                            # int(A*x + B) as float; softmax averaging over
                            # S=2048 plus denominator cancellation shrinks
                            # the ~3% sawtooth to ~0.1-0.3% on the output.
                            pti = ptp.tile([128, 2048], mybir.dt.int32,
                                           tag="pt", name=f"pti{h}_{pi}")
                            with nc.allow_low_precision(reason="fast exp"):
                                nc.vector.tensor_scalar(
                                    out=pti, in0=sps,
                                    scalar1=float(SCALE * 12102203.1616),
                                    scalar2=1064866805.0,
                                    op0=OP.mult, op1=OP.add)
                            # walrus requires an f32r-rounding producer for
                            # f32r matmul operands; a converting copy is one
                            pt = ptp.tile([128, 2048], F32R, tag="pt",
                                          name=f"ptr{h}_{pi}")
                            nc.vector.tensor_copy(pt, pti.bitcast(F32))
                        else:
                            pt = ptp.tile([128, 2048], F32R, tag="pt",
                                          name=f"pt{h}_{pi}")
                            nc.scalar.activation(pt, sps, AF.Exp,
                                                 scale=float(SCALE))
                        for sub in range(2):
                            pts[2 * pi + sub] = (pt, 1024 * sub)
                        if "a" in stages:
                            while nxt <= 2 * pi + 1 - LOOK:
                                pv_chunk(nxt)
                                nxt += 1
                    if "a" not in stages:
                        pts.clear()
                        continue
                    while nxt < ST:
                        pv_chunk(nxt)
                        nxt += 1

                    # normalize by the denominator row
                    for tb in range(n_tb):
                        cmb = ovp.tile([HD + 1, MOV], F32, tag="ov",
                                       name=f"cb{h}_{tb}")
                        nc.vector.tensor_copy(cmb, pvs[tb])
                        rt = rtp.tile([65, MOV], F32R, tag="rt",
                                      name=f"rt{h}_{tb}")
                        with nc.allow_low_precision(
                                reason="recip feeds f32r matmul"):
                            nc.vector.reciprocal(rt[64:65, :],
                                                 cmb[HD:HD + 1, :])
                        bc = (aux if v_overlap else pvp).tile(
                            [64, MOV], F32, tag="aux" if v_overlap else "pv",
                            name=f"bcm{h}_{tb}")
                        nc.tensor.matmul(bc, ones_r[64:65, :], rt[64:65, :],
                                         start=True, stop=True)
                        nc.vector.tensor_tensor(
                            out=ot[pr:pr + 64, ch, MOV * tb:MOV * (tb + 1)],
                            in0=cmb[0:HD, :], in1=bc, op=OP.mult)

                # ---- output projection (into one staging tile, 1 store) ----
                for tt in range(TSH // 128 if "o" in stages else 0):
                    mm = stp.tile([128, D], F32, tag="st", name=f"omm{tt}")
                    for c in range(DC):
                        nc.tensor.matmul(mm, ot[:, c, 128 * tt:128 * (tt + 1)],
                                         wts["wo"][:, c, :],
                                         start=(c == 0), stop=(c == DC - 1))
                    nc.vector.tensor_tensor(out=outsb[:, tt, :], in0=mm,
                                            in1=bo_bc, op=OP.add)
                if "o" in stages:
                    nc.sync.dma_start(
                        out=out_d.ap().rearrange("(a p) i -> p a i", p=128),
                        in_=outsb)

    nc.compile()
    return nc


_CACHE = {}


def _get_nc():
    if "nc" not in _CACHE:
        _CACHE["nc"] = build_nc()
    return _CACHE["nc"]


def kernel(query, key, value, Wq, bq, Wk, bk, Wv, bv, Wo, bo):
    f = lambda x: np.ascontiguousarray(np.asarray(x, dtype=np.float32))
    query, key, value = f(query), f(key), f(value)
    shared = {"wq": f(Wq), "wk": f(Wk), "wv": f(Wv), "wo": f(Wo),
              "bq": f(bq), "bk": f(bk), "bv": f(bv), "bo": f(bo)}
    in_maps = []
    for c in range(NCORES):
        b, th = divmod(c, 2)
        in_maps.append({
            "q": query[b, th * TSH:(th + 1) * TSH, :],
            "k": key[b], "v": value[b], **shared,
        })
    nc = _get_nc()
    res = run_bass_kernel_spmd(nc, in_maps, core_ids=list(range(NCORES)))
    out = np.empty((B, T, D), dtype=np.float32)
    for c in range(NCORES):
        b, th = divmod(c, 2)
        out[b, th * TSH:(th + 1) * TSH, :] = res.results[c]["out"]
    return out
